# revision 34
# baseline (speedup 1.0000x reference)
"""Trainium2 Bass kernel for nn_DepthWiseConv_AConnect (depthwise 3x3 conv with
per-pool multiplicative weight/bias noise, followed by 8-bit LQuant).

Strategy (8 NeuronCores, data-parallel over the pool axis):
  - Core p handles pool group p: 8 images, Werr[p], Berr[p].
  - X ships fp16 (exact products in fp32 PSUM), outputs ship int8.
  - Per 9-output-row PSUM bank: ScalarE seeds the bank with
    tap0*x + bias (activation with per-partition scale/bias), then the
    TensorEngine accumulates taps 1..8 as diagonal matmuls (start=False).
  - A single fused custom-DVE op does the entire LQuant: adds the
    1.5*2^23 magic (RNE round to the integer grid), clips to
    [MAGIC-127, MAGIC+127], subtracts the magic back out, and emits int8.
  - Host divides by 127 and transposes back to NHWC.
"""
import re
import sys

import numpy as np

try:
    import concourse.bacc as bacc_mod
except ImportError:
    sys.path.insert(0, "/opt/trn_rl_repo")
    import concourse.bacc as bacc_mod

import concourse.mybir as mybir
from concourse.tile import TileContext
from concourse.tile_rust import add_dep_helper
from concourse.bass_utils import run_bass_kernel_spmd
from contextlib import ExitStack

POOL = 8
NB = 8            # images per pool group (64 / 8)
H = W = 56
HO = WO = 54
C = 256
NCH = 2           # channel chunks of 128
NPIX = H * W      # 3136
NOUT = HO * WO    # 2916
BANKN = 486       # output pixels per psum bank (9 rows x 54)
HALFN = 3 * BANKN  # 1458 output pixels per half (27 rows)
MAGIC = 12582912.0  # 1.5 * 2^23
S = 127.0

f32 = mybir.dt.float32
f16 = mybir.dt.float16
i8 = mybir.dt.int8

_cached = {}


def _register_quant_ops():
    """Register the fused LQuant custom-DVE ops (idempotent)."""
    from concourse import dve_ops
    from concourse.dve_spec import Spec, Src0, Src1, C0, C1, maxx, minn
    from concourse.bass import dve_ver_for

    def reg(name, spec):
        for op in dve_ops.OPS:
            if op.name == name:
                return op
        op = dve_ops.DveOp(name, spec, subdim=False, uops_sha={})
        dve_ops.OPS.append(op)
        dve_ops.CUSTOM_DVE_SPECS[name] = spec
        dve_ops._SUB_OPCODE_FOR_NAME[name] = (
            dve_ops._CUSTOM_DVE_ROW_BASE + len(dve_ops.OPS) - 1
        )
        ver = dve_ver_for("TRN2")
        try:
            op.compile(ver)
        except ValueError as e:  # harvest the computed sha from the message
            m = re.findall(r'="([0-9a-f]+)"', str(e))
            assert m, f"could not parse uops sha from: {e}"
            op.uops_sha[ver] = m[-1]
            dve_ops._COMPILE_CACHE.pop((name, ver), None)
            op.compile(ver)
        return op

    def _ref_q(in0, in1, s0, s1, imm2):
        t = in0.astype(np.float32) + np.float32(s0)
        t = np.minimum(np.maximum(t, np.float32(s1)), np.float32(imm2))
        return t - np.float32(s0)

    def _ref_qa(in0, in1, s0, s1, imm2):
        t = in0.astype(np.float32) + in1.astype(np.float32)
        t = (t + np.float32(s0)) - np.float32(s0)
        return np.minimum(np.maximum(t, np.float32(-s1)), np.float32(s1))

    from concourse.dve_spec import Zero, C2
    q = reg("LQUANT_MAGIC_ANT", Spec(
        body=minn(maxx(Src0 + C0, C1), C2) - C0,
        reference=_ref_q))
    qa = reg("LQUANT_MAGIC_ACC_ANT", Spec(
        body=minn(maxx(((Src0 + Src1) + C0) - C0, Zero - C1), C1),
        reference=_ref_qa))
    return q, qa


def _build():
    quant_op, quant_acc_op = _register_quant_ops()

    nc = bacc_mod.Bacc()
    xt = nc.dram_tensor("xt", [NB, NCH, 128, NPIX], f16, kind="ExternalInput")
    wdg = nc.dram_tensor("wdg", [NCH, 9, 128, 128], f16, kind="ExternalInput")
    # per-channel scalars: [tap7 weight, tap8 weight, bias]
    wsc = nc.dram_tensor("wsc", [NCH, 128, 3], f32, kind="ExternalInput")
    out = nc.dram_tensor("out", [NB, NCH, 128, NOUT], i8, kind="ExternalOutput")

    with TileContext(nc) as tc, ExitStack() as ctx:
        consts = ctx.enter_context(tc.tile_pool(name="consts", bufs=1))
        xpool = ctx.enter_context(tc.tile_pool(name="xpool", bufs=3))
        opool = ctx.enter_context(tc.tile_pool(name="opool", bufs=3))
        vpool = ctx.enter_context(tc.tile_pool(name="vpool", bufs=3))
        pspool = ctx.enter_context(tc.tile_pool(name="pspool", bufs=2, space="PSUM"))

        ws = consts.tile([128, NCH, 9, 128], f16)
        nc.sync.dma_start(out=ws, in_=wdg.rearrange("q t k m -> k q t m"))
        sc = consts.tile([128, NCH, 3], f32)
        nc.sync.dma_start(out=sc, in_=wsc.rearrange("q k s -> k q s"))

        prev = None
        for n in range(NB):
            for q in range(NCH):
                xs = xpool.tile([128, NPIX], f16, tag="xs")
                nc.sync.dma_start(out=xs, in_=xt[n, q])
                xr = xs.rearrange("p (h w) -> p h w", w=W)
                w7 = sc[:, q, 0:1]
                w8 = sc[:, q, 1:2]
                bv = sc[:, q, 2:3]
                for h in range(2):
                    # taps 7 (2,1) and 8 (2,2) + bias on the DVE in fp16
                    r0 = 27 * h + 2
                    tmp1 = vpool.tile([128, 27, WO], f16, tag="tmp1")
                    nc.vector.tensor_scalar(
                        out=tmp1, in0=xr[:, r0:r0 + 27, 1:1 + WO],
                        scalar1=w7, scalar2=bv,
                        op0=mybir.AluOpType.mult, op1=mybir.AluOpType.add)
                    tmp2 = vpool.tile([128, 27, WO], f16, tag="tmp2")
                    nc.vector.tensor_scalar(
                        out=tmp2, in0=xr[:, r0:r0 + 27, 2:2 + WO],
                        scalar1=w8, scalar2=None,
                        op0=mybir.AluOpType.mult)
                    acc = vpool.tile([128, 27, WO], f16, tag="acc")
                    nc.vector.tensor_tensor(out=acc, in0=tmp1, in1=tmp2,
                                            op=mybir.AluOpType.add)
                    ps = pspool.tile([128, 3, 512], f32, tag="ps")
                    for b3 in range(3):
                        bk = 3 * h + b3
                        for t in range(7):
                            i, j = divmod(t, 3)
                            rhs = xr[:, 9 * bk + i: 9 * bk + i + 9, j: j + WO]
                            nc.tensor.matmul(ps[:, b3, 0:BANKN],
                                             lhsT=ws[:, q, t, :], rhs=rhs,
                                             start=(t == 0), stop=(t == 6),
                                             skip_group_check=True)
                    # software pipeline: quantize the PREVIOUS unit now, so
                    # the DVE's tap work above filled the PE wait time.
                    if prev is not None:
                        pps, pacc, pn, pq, ph = prev
                        ot = opool.tile([128, 3, BANKN], i8, tag="ot")
                        nc.vector._custom_dve(
                            quant_acc_op, out=ot, in0=pps[:, :, 0:BANKN],
                            in1=pacc.rearrange("p (c r) w -> p c (r w)", r=9),
                            s0=MAGIC, s1=S)
                        nc.sync.dma_start(
                            out=out[pn, pq][:, HALFN * ph: HALFN * (ph + 1)]
                            .rearrange("p (a b) -> p a b", b=BANKN),
                            in_=ot)
                    prev = (ps, acc, n, q, h)

        pps, pacc, pn, pq, ph = prev
        ot = opool.tile([128, 3, BANKN], i8, tag="ot")
        nc.vector._custom_dve(
            quant_acc_op, out=ot, in0=pps[:, :, 0:BANKN],
            in1=pacc.rearrange("p (c r) w -> p c (r w)", r=9),
            s0=MAGIC, s1=S)
        nc.sync.dma_start(
            out=out[pn, pq][:, HALFN * ph: HALFN * (ph + 1)]
            .rearrange("p (a b) -> p a b", b=BANKN),
            in_=ot)

    nc.finalize()
    return nc


def kernel(X, W, bias, Werr, Berr, _trace=False):
    X = np.asarray(X, np.float32)
    W = np.asarray(W, np.float32)
    bias = np.asarray(bias, np.float32)
    Werr = np.asarray(Werr, np.float32)
    Berr = np.asarray(Berr, np.float32)

    if "nc" not in _cached:
        _cached["nc"] = _build()
    nc = _cached["nc"]

    Xh = X.astype(np.float16)  # [64, 56, 56, 256]
    w3 = W[..., 0]             # [3, 3, 256]
    we3 = Werr[..., 0]         # [8, 3, 3, 256]

    in_maps = []
    for p in range(POOL):
        xp = Xh[p * NB:(p + 1) * NB].reshape(NB, NPIX, C)
        xp = np.ascontiguousarray(xp.transpose(0, 2, 1)).reshape(NB, NCH, 128, NPIX)

        w_eff = np.float32(S) * w3 * we3[p]  # [3, 3, 256] fp32
        w_eff16 = w_eff.astype(np.float16)
        wdg = np.zeros((NCH, 9, 128, 128), np.float16)
        for q in range(NCH):
            for t in range(9):
                i, j = divmod(t, 3)
                np.fill_diagonal(wdg[q, t], w_eff16[i, j, 128 * q:128 * (q + 1)])

        b_eff = (np.float32(S) * bias * Berr[p]).astype(np.float32)
        wsc = np.stack([w_eff[2, 1].astype(np.float32),
                        w_eff[2, 2].astype(np.float32), b_eff],
                       axis=-1).reshape(NCH, 128, 3)
        in_maps.append({"xt": xp, "wdg": wdg, "wsc": wsc})

    res = run_bass_kernel_spmd(nc, in_maps, core_ids=list(range(POOL)),
                               trace=_trace)
    if _trace:
        _cached["last_result"] = res

    outs = []
    for p in range(POOL):
        o = res.results[p]["out"].astype(np.float32)  # [NB, NCH, 128, NOUT] int8
        o = o / np.float32(S)
        o = o.reshape(NB, C, HO, WO).transpose(0, 2, 3, 1)  # NHWC
        outs.append(o)
    return np.ascontiguousarray(np.concatenate(outs, axis=0).astype(np.float32))


# revision 41
# speedup vs baseline: 1.1653x; 1.1653x over previous
"""Trainium2 Bass kernel for nn_DepthWiseConv_AConnect (depthwise 3x3 conv with
per-pool multiplicative weight/bias noise, followed by 8-bit LQuant).

Strategy (8 NeuronCores, data-parallel over the pool axis):
  - Core p handles pool group p: 8 images, Werr[p], Berr[p].
  - X ships fp16 (exact products in fp32 PSUM), outputs ship int8.
  - Per 9-output-row PSUM bank: ScalarE seeds the bank with
    tap0*x + bias (activation with per-partition scale/bias), then the
    TensorEngine accumulates taps 1..8 as diagonal matmuls (start=False).
  - A single fused custom-DVE op does the entire LQuant: adds the
    1.5*2^23 magic (RNE round to the integer grid), clips to
    [MAGIC-127, MAGIC+127], subtracts the magic back out, and emits int8.
  - Host divides by 127 and transposes back to NHWC.
"""
import re
import sys

import numpy as np

try:
    import concourse.bacc as bacc_mod
except ImportError:
    sys.path.insert(0, "/opt/trn_rl_repo")
    import concourse.bacc as bacc_mod

import concourse.mybir as mybir
from concourse.tile import TileContext
from concourse.bass_utils import run_bass_kernel_spmd
from contextlib import ExitStack

POOL = 8
NB = 8            # images per pool group (64 / 8)
H = W = 56
HO = WO = 54
C = 256
NCH = 2           # channel chunks of 128
NPIX = H * W      # 3136
NOUT = HO * WO    # 2916
BANKN = 486       # output pixels per psum bank (9 rows x 54)
HALFN = 3 * BANKN  # 1458 output pixels per half (27 rows)
MAGIC = 12582912.0  # 1.5 * 2^23
S = 127.0

f32 = mybir.dt.float32
f16 = mybir.dt.float16
i8 = mybir.dt.int8

_cached = {}


def _register_quant_ops():
    """Register the fused LQuant custom-DVE ops (idempotent)."""
    from concourse import dve_ops
    from concourse.dve_spec import Spec, Src0, Src1, C0, C1, maxx, minn
    from concourse.bass import dve_ver_for

    def reg(name, spec):
        for op in dve_ops.OPS:
            if op.name == name:
                return op
        op = dve_ops.DveOp(name, spec, subdim=False, uops_sha={})
        dve_ops.OPS.append(op)
        dve_ops.CUSTOM_DVE_SPECS[name] = spec
        dve_ops._SUB_OPCODE_FOR_NAME[name] = (
            dve_ops._CUSTOM_DVE_ROW_BASE + len(dve_ops.OPS) - 1
        )
        ver = dve_ver_for("TRN2")
        try:
            op.compile(ver)
        except ValueError as e:  # harvest the computed sha from the message
            m = re.findall(r'="([0-9a-f]+)"', str(e))
            assert m, f"could not parse uops sha from: {e}"
            op.uops_sha[ver] = m[-1]
            dve_ops._COMPILE_CACHE.pop((name, ver), None)
            op.compile(ver)
        return op

    def _ref_q(in0, in1, s0, s1, imm2):
        t = in0.astype(np.float32) + np.float32(s0)
        t = np.minimum(np.maximum(t, np.float32(s1)), np.float32(imm2))
        return t - np.float32(s0)

    def _ref_qa(in0, in1, s0, s1, imm2):
        t = in0.astype(np.float32) + in1.astype(np.float32)
        t = (t + np.float32(s0)) - np.float32(s0)
        return np.minimum(np.maximum(t, np.float32(-s1)), np.float32(s1))

    from concourse.dve_spec import Zero, C2
    q = reg("LQUANT_MAGIC_ANT", Spec(
        body=minn(maxx(Src0 + C0, C1), C2) - C0,
        reference=_ref_q))
    qa = reg("LQUANT_MAGIC_ACC_ANT", Spec(
        body=minn(maxx(((Src0 + Src1) + C0) - C0, Zero - C1), C1),
        reference=_ref_qa))
    return q, qa


def _build():
    quant_op, quant_acc_op = _register_quant_ops()

    nc = bacc_mod.Bacc()
    xt = nc.dram_tensor("xt", [NB, NCH, 128, NPIX], f16, kind="ExternalInput")
    # weights pre-arranged partition-major on host for contiguous DMA
    wdg = nc.dram_tensor("wdg", [128, NCH, 7, 128], f16, kind="ExternalInput")
    # per-channel scalars: [tap7 weight, tap8 weight, bias]
    wsc = nc.dram_tensor("wsc", [128, NCH, 3], f32, kind="ExternalInput")
    out = nc.dram_tensor("out", [NB, NCH, 128, NOUT], i8, kind="ExternalOutput")

    with TileContext(nc) as tc, ExitStack() as ctx:
        consts = ctx.enter_context(tc.tile_pool(name="consts", bufs=1))
        xpool = ctx.enter_context(tc.tile_pool(name="xpool", bufs=3))
        opool = ctx.enter_context(tc.tile_pool(name="opool", bufs=3))
        vpool = ctx.enter_context(tc.tile_pool(name="vpool", bufs=3))
        pspool = ctx.enter_context(tc.tile_pool(name="pspool", bufs=2, space="PSUM"))

        ws = consts.tile([128, NCH, 7, 128], f16)
        nc.sync.dma_start(out=ws, in_=wdg[:, :, :, :])
        sc = consts.tile([128, NCH, 3], f32)
        nc.sync.dma_start(out=sc, in_=wsc[:, :, :])

        for n in range(NB):
            for q in range(NCH):
                xs = xpool.tile([128, NPIX], f16, tag="xs")
                nc.sync.dma_start(out=xs, in_=xt[n, q])
                xr = xs.rearrange("p (h w) -> p h w", w=W)
                w7 = sc[:, q, 0:1]
                w8 = sc[:, q, 1:2]
                bv = sc[:, q, 2:3]
                for h in range(2):
                    # taps 7 (2,1) and 8 (2,2) + bias on the DVE in fp16
                    r0 = 27 * h + 2
                    tmp1 = vpool.tile([128, 27, WO], f16, tag="tmp1")
                    nc.vector.tensor_scalar(
                        out=tmp1, in0=xr[:, r0:r0 + 27, 1:1 + WO],
                        scalar1=w7, scalar2=bv,
                        op0=mybir.AluOpType.mult, op1=mybir.AluOpType.add)
                    tmp2 = vpool.tile([128, 27, WO], f16, tag="tmp2")
                    nc.vector.tensor_scalar(
                        out=tmp2, in0=xr[:, r0:r0 + 27, 2:2 + WO],
                        scalar1=w8, scalar2=None,
                        op0=mybir.AluOpType.mult)
                    acc = vpool.tile([128, 27, WO], f16, tag="acc")
                    nc.vector.tensor_tensor(out=acc, in0=tmp1, in1=tmp2,
                                            op=mybir.AluOpType.add)
                    ps = pspool.tile([128, 3, 512], f32, tag="ps")
                    for b3 in range(3):
                        bk = 3 * h + b3
                        for t in range(7):
                            i, j = divmod(t, 3)
                            rhs = xr[:, 9 * bk + i: 9 * bk + i + 9, j: j + WO]
                            nc.tensor.matmul(ps[:, b3, 0:BANKN],
                                             lhsT=ws[:, q, t, :], rhs=rhs,
                                             start=(t == 0), stop=(t == 6),
                                             skip_group_check=True)
                    ot = opool.tile([128, 3, BANKN], i8, tag="ot")
                    nc.vector._custom_dve(
                        quant_acc_op, out=ot, in0=ps[:, :, 0:BANKN],
                        in1=acc.rearrange("p (c r) w -> p c (r w)", r=9),
                        s0=MAGIC, s1=S)
                    nc.sync.dma_start(
                        out=out[n, q][:, HALFN * h: HALFN * (h + 1)]
                        .rearrange("p (a b) -> p a b", b=BANKN),
                        in_=ot)

    nc.finalize()
    return nc


def kernel(X, W, bias, Werr, Berr, _trace=False):
    X = np.asarray(X, np.float32)
    W = np.asarray(W, np.float32)
    bias = np.asarray(bias, np.float32)
    Werr = np.asarray(Werr, np.float32)
    Berr = np.asarray(Berr, np.float32)

    if "nc" not in _cached:
        _cached["nc"] = _build()
    nc = _cached["nc"]

    Xh = X.astype(np.float16)  # [64, 56, 56, 256]
    w3 = W[..., 0]             # [3, 3, 256]
    we3 = Werr[..., 0]         # [8, 3, 3, 256]

    in_maps = []
    for p in range(POOL):
        xp = Xh[p * NB:(p + 1) * NB].reshape(NB, NPIX, C)
        xp = np.ascontiguousarray(xp.transpose(0, 2, 1)).reshape(NB, NCH, 128, NPIX)

        w_eff = np.float32(S) * w3 * we3[p]  # [3, 3, 256] fp32
        w_eff16 = w_eff.astype(np.float16)
        wdg = np.zeros((NCH, 7, 128, 128), np.float16)
        for q in range(NCH):
            for t in range(7):
                i, j = divmod(t, 3)
                np.fill_diagonal(wdg[q, t], w_eff16[i, j, 128 * q:128 * (q + 1)])
        wdg = np.ascontiguousarray(wdg.transpose(2, 0, 1, 3))  # [128,NCH,7,128]

        b_eff = (np.float32(S) * bias * Berr[p]).astype(np.float32)
        wsc = np.stack([w_eff[2, 1].astype(np.float32),
                        w_eff[2, 2].astype(np.float32), b_eff],
                       axis=-1).reshape(NCH, 128, 3)
        wsc = np.ascontiguousarray(wsc.transpose(1, 0, 2))  # [128, NCH, 3]
        in_maps.append({"xt": xp, "wdg": wdg, "wsc": wsc})

    res = run_bass_kernel_spmd(nc, in_maps, core_ids=list(range(POOL)),
                               trace=_trace)
    if _trace:
        _cached["last_result"] = res

    outs = []
    for p in range(POOL):
        o = res.results[p]["out"].astype(np.float32)  # [NB, NCH, 128, NOUT] int8
        o = o / np.float32(S)
        o = o.reshape(NB, C, HO, WO).transpose(0, 2, 3, 1)  # NHWC
        outs.append(o)
    return np.ascontiguousarray(np.concatenate(outs, axis=0).astype(np.float32))
